# revision 16
# baseline (speedup 1.0000x reference)
"""ArrowLoRA MoE routing kernel for 8 TRN2 NeuronCores.

Math (per token t of 8192, F=2048, E=16 experts, R=16, O=2048):
    sim   = |x @ protos.T|                       (t, E)
    coeff = softmax(top4-masked sim)             (t, E)
    z     = x @ Acat.T                           (t, E*R)   Acat = A_stack.reshape(256, F)
    delta = scaling * (coeff-weighted z) @ Bcat  (t, O)     Bcat = B_stack.transpose(0,2,1).reshape(256, O)

Strategy: token-parallel across 8 cores (1024 tokens each), weights
replicated, no collectives. Host pre-transposes/casts x to fp16 (bf16 is
NOT enough precision for the top-4 routing: it flips picks for ~0.7% of
tokens and fails the 2e-2 gate; fp16 misroutes only ~0.05% and lands at
rel_err ~7e-3 including an fp16 output).

Per core: one fused matmul per 128-token tile produces z and sim together
(moving operand = [Acat.T | protos.T], 272 cols); top-4 via the DVE top-8
`max` op; softmax on batched [128, 8, 16] tiles; zw = z * coeff via a
broadcast AP; PE-transpose of zw to put E*R on partitions; second matmul
against Bcat (scaling folded in on host); fp16 output upcast on host.
"""

import os

import numpy as np

import concourse.bass as bass
import concourse.mybir as mybir
from concourse import bacc
from concourse.bass import ts
from concourse.bass_utils import run_bass_kernel_spmd
from concourse.tile import TileContext

# Problem shape (hardcoded per spec).
B, S, F, E, R, O = 4, 2048, 2048, 16, 16, 2048
TOPK = 4
NCORES = 8
T = B * S                  # 8192 tokens
TPC = T // NCORES          # 1024 tokens per core
NT = TPC // 128            # 8 token tiles per core
FCH = F // 128             # 16 F chunks (contraction)
ER = E * R                 # 256
ERCH = ER // 128           # 2 er chunks
WCOLS = ER + E             # 272: [z cols | sim cols]

F16 = mybir.dt.float16
F32 = mybir.dt.float32

_CACHE = {}

LAST_RESULTS = None  # BassKernelResults of the most recent run (for test.py)


def build_nc():
    # Bacc (not raw Bass): its compile passes legalize sync waits to the
    # 1-wait-per-instruction HW limit (move_matmul_waits_to_ldweights +
    # generate_event_semaphores) — raw Bass graphs with recycled PSUM slots
    # fail walrus codegen with "Too many sync wait commands".
    nc = bacc.Bacc(target_bir_lowering=False)

    xT = nc.declare_dram_parameter("xT", [FCH, 128, TPC], F16, isOutput=False)
    W = nc.declare_dram_parameter("W", [FCH, 128, WCOLS], F16, isOutput=False)
    Bc = nc.declare_dram_parameter("Bc", [ERCH, 128, O], F16, isOutput=False)
    ident = nc.declare_dram_parameter("ident", [128, 128], F16, isOutput=False)
    out = nc.declare_dram_parameter("out", [TPC, O], F16, isOutput=True)

    with TileContext(nc) as tc:
        with (
            tc.tile_pool(name="weights", bufs=1) as wpool,
            tc.tile_pool(name="xdata", bufs=1) as xpool,
            tc.tile_pool(name="zdata", bufs=1) as zpool,
            tc.tile_pool(name="small", bufs=1) as small,
            tc.tile_pool(name="work", bufs=3) as work,
            tc.tile_pool(name="psum_zs", bufs=2, space="PSUM") as psum_zs,
            tc.tile_pool(name="psum_t", bufs=2, space="PSUM") as psum_t,
            tc.tile_pool(name="psum_d", bufs=4, space="PSUM") as psum_d,
        ):
            # ---- input DMAs (chunked so matmuls can start early) ----
            x_sb = []
            w_sb = []
            for c in range(FCH):
                xc = xpool.tile([128, TPC], F16, name=f"x_sb{c}", tag=f"x{c}")
                nc.sync.dma_start(out=xc, in_=xT[c])
                x_sb.append(xc)
                wc = xpool.tile([128, WCOLS], F16, name=f"w_sb{c}", tag=f"w{c}")
                nc.sync.dma_start(out=wc, in_=W[c])
                w_sb.append(wc)
            id_sb = wpool.tile([128, 128], F16, name="id_sb")
            nc.sync.dma_start(out=id_sb, in_=ident[:, :])
            bc_sb = wpool.tile([128, ERCH, O], F16, name="bc_sb")
            nc.sync.dma_start(out=bc_sb, in_=Bc[:, :, :].rearrange("c p o -> p c o"))

            # ---- routing stat tiles (batched over the 8 token tiles) ----
            sim_all = small.tile([128, NT, E], F32, name="sim_all")
            m8_all = small.tile([128, NT, 8], F32, name="m8_all")
            shifted = small.tile([128, NT, E], F32, name="shifted")
            e_all = small.tile([128, NT, E], F32, name="e_all")
            ge_all = small.tile([128, NT, E], F32, name="ge_all")
            em_all = small.tile([128, NT, E], F32, name="em_all")
            den_all = small.tile([128, NT], F32, name="den_all")
            rcp_all = small.tile([128, NT], F32, name="rcp_all")
            coeff_all = small.tile([128, NT, E], F32, name="coeff_all")

            # ---- phase 1: fused z+sim matmul per token tile ----
            z_sb = []
            for i in range(NT):
                zs = psum_zs.tile([128, WCOLS], F32, name=f"zs{i}", tag="zs")
                for c in range(FCH):
                    nc.tensor.matmul(
                        zs,
                        lhsT=x_sb[c][:, ts(i, 128)],
                        rhs=w_sb[c],
                        start=(c == 0),
                        stop=(c == FCH - 1),
                    )
                # |sim| into the batched stat tile; z to SBUF fp16
                nc.scalar.activation(
                    sim_all[:, i, :], zs[:, ER:WCOLS],
                    mybir.ActivationFunctionType.Abs,
                )
                zc = zpool.tile([128, ER], F16, name=f"z_sb{i}", tag=f"z{i}")
                nc.vector.tensor_copy(out=zc, in_=zs[:, 0:ER])
                z_sb.append(zc)
                nc.vector.max(m8_all[:, i, :], sim_all[:, i, :])

            # ---- phase 2: batched top-4 softmax ----
            m1_bc = m8_all[:, :, 0:1].to_broadcast([128, NT, E])
            th_bc = m8_all[:, :, 3:4].to_broadcast([128, NT, E])
            nc.vector.tensor_tensor(
                shifted, sim_all, m1_bc, mybir.AluOpType.subtract
            )
            nc.scalar.activation(
                e_all, shifted, mybir.ActivationFunctionType.Exp
            )
            nc.vector.tensor_tensor(ge_all, sim_all, th_bc, mybir.AluOpType.is_ge)
            nc.vector.tensor_tensor(em_all, e_all, ge_all, mybir.AluOpType.mult)
            nc.vector.tensor_reduce(
                den_all, em_all, axis=mybir.AxisListType.X, op=mybir.AluOpType.add
            )
            nc.vector.reciprocal(rcp_all, den_all)
            nc.vector.tensor_tensor(
                coeff_all, em_all,
                rcp_all[:, :, None].to_broadcast([128, NT, E]),
                mybir.AluOpType.mult,
            )

            # ---- phase 3: weight z, transpose, second matmul, store ----
            for i in range(NT):
                zw = work.tile([128, E, R], F16, name=f"zw{i}", tag="zw")
                nc.vector.tensor_tensor(
                    zw,
                    z_sb[i].rearrange("p (e r) -> p e r", r=R),
                    coeff_all[:, i, :, None].to_broadcast([128, E, R]),
                    mybir.AluOpType.mult,
                )
                zwT = work.tile([128, ERCH, 128], F16, name=f"zwT{i}", tag="zwT")
                for h in range(ERCH):
                    tp = psum_t.tile([128, 128], F16, name=f"tp{i}_{h}", tag="tp")
                    nc.tensor.transpose(
                        tp, zw.rearrange("p e r -> p (e r)")[:, ts(h, 128)], id_sb
                    )
                    nc.vector.tensor_copy(out=zwT[:, h, :], in_=tp)

                out_sb = work.tile([128, O], F16, name=f"out_sb{i}", tag="out_sb")
                for q in range(4):
                    dq = psum_d.tile([128, 512], F32, name=f"d{i}_{q}", tag="d")
                    for ch in range(ERCH):
                        nc.tensor.matmul(
                            dq,
                            lhsT=zwT[:, ch, :],
                            rhs=bc_sb[:, ch, ts(q, 512)],
                            start=(ch == 0),
                            stop=(ch == ERCH - 1),
                        )
                    if q % 2 == 0:
                        nc.scalar.activation(
                            out_sb[:, ts(q, 512)], dq,
                            mybir.ActivationFunctionType.Copy,
                        )
                    else:
                        nc.vector.tensor_copy(out=out_sb[:, ts(q, 512)], in_=dq)
                nc.sync.dma_start(out=out[ts(i, 128), :], in_=out_sb)

    nc.finalize()  # runs Bacc.compile(): reg alloc + sync-wait legalization
    return nc


def _host_prep(x, prototypes, A_stack, B_stack, scaling):
    tok = np.ascontiguousarray(x.reshape(T, F))

    Acat = A_stack.reshape(ER, F)
    Wh = np.concatenate([Acat.T, prototypes.T], axis=1)        # (F, 272)
    Wh = Wh.reshape(FCH, 128, WCOLS).astype(np.float16)

    Bcat = (B_stack.transpose(0, 2, 1).reshape(ER, O) * float(scaling))
    Bch = Bcat.reshape(ERCH, 128, O).astype(np.float16)

    identh = np.eye(128, dtype=np.float16)

    in_maps = []
    for core in range(NCORES):
        shard = tok[core * TPC:(core + 1) * TPC]               # (TPC, F)
        xTh = shard.T.reshape(FCH, 128, TPC).astype(np.float16)
        in_maps.append({
            "xT": np.ascontiguousarray(xTh),
            "W": Wh,
            "Bc": Bch,
            "ident": identh,
        })
    return in_maps


def _setup_axon_tracing():
    """Make trace=True work in this container: register the NTFF profile
    hook that the image's antenv lacks, and neuter upload_artifacts (no
    artifact store here). Best-effort — failures just disable tracing."""
    import sys
    import types

    import concourse.bass_utils as bu

    bu.upload_artifacts = lambda tmpdir: "local://" + tmpdir
    try:
        from antenv.axon_hooks import get_axon_ntff_profile_hook  # noqa: F401
        return
    except ImportError:
        pass
    import antenv
    from trn_agent_boot.trn_boot import _ntff_profile_via_ctypes

    mod = types.ModuleType("antenv.axon_hooks")
    state = {"hook": _ntff_profile_via_ctypes("/opt/axon/libaxon_pjrt.so")}
    mod.set_axon_ntff_profile_hook = lambda h: state.__setitem__("hook", h)
    mod.get_axon_ntff_profile_hook = lambda: state["hook"]
    antenv.axon_hooks = mod
    sys.modules["antenv.axon_hooks"] = mod


def kernel(x, prototypes, A_stack, B_stack, scaling, top_k):
    global LAST_RESULTS
    assert int(top_k) == TOPK, f"kernel hardcodes top_k={TOPK}, got {top_k}"
    assert x.shape == (B, S, F)

    if "nc" not in _CACHE:
        _CACHE["nc"] = build_nc()
    nc = _CACHE["nc"]

    in_maps = _host_prep(
        np.asarray(x, dtype=np.float32),
        np.asarray(prototypes, dtype=np.float32),
        np.asarray(A_stack, dtype=np.float32),
        np.asarray(B_stack, dtype=np.float32),
        np.asarray(scaling, dtype=np.float32),
    )

    trace = os.environ.get("KERNEL_TRACE", "0") == "1"
    if trace:
        try:
            _setup_axon_tracing()
        except Exception as e:  # tracing is optional; never fail the run
            print(f"tracing setup failed ({e}); running without trace")
            trace = False
    res = run_bass_kernel_spmd(nc, in_maps, core_ids=list(range(NCORES)), trace=trace)
    LAST_RESULTS = res

    outs = [res.results[i]["out"] for i in range(NCORES)]
    full = np.concatenate(outs, axis=0).astype(np.float32)
    return full.reshape(B, S, O)


# revision 19
# speedup vs baseline: 1.0912x; 1.0912x over previous
"""ArrowLoRA MoE routing kernel for 8 TRN2 NeuronCores.

Math (per token t of 8192, F=2048, E=16 experts, R=16, O=2048):
    sim   = |x @ protos.T|                       (t, E)
    coeff = softmax(top4-masked sim)             (t, E)
    z     = x @ Acat.T                           (t, E*R)   Acat = A_stack.reshape(256, F)
    delta = scaling * (coeff-weighted z) @ Bcat  (t, O)     Bcat = B_stack.transpose(0,2,1).reshape(256, O)

Strategy: token-parallel across 8 cores (1024 tokens each), weights
replicated, no collectives. Host pre-transposes/casts x to fp16 (bf16 is
NOT enough precision for the top-4 routing: it flips picks for ~0.7% of
tokens and fails the 2e-2 gate; fp16 misroutes only ~0.05% and lands at
rel_err ~7e-3 including an fp16 output).

Per core: one fused matmul per 128-token tile produces z and sim together
(moving operand = [Acat.T | protos.T], 272 cols); top-4 via the DVE top-8
`max` op; softmax on batched [128, 8, 16] tiles; zw = z * coeff via a
broadcast AP; PE-transpose of zw to put E*R on partitions; second matmul
against Bcat (scaling folded in on host); fp16 output upcast on host.
"""

import os

import numpy as np

import concourse.bass as bass
import concourse.mybir as mybir
from concourse import bacc
from concourse.bass import ts
from concourse.bass_utils import run_bass_kernel_spmd
from concourse.tile import TileContext

# Problem shape (hardcoded per spec).
B, S, F, E, R, O = 4, 2048, 2048, 16, 16, 2048
TOPK = 4
NCORES = 8
T = B * S                  # 8192 tokens
TPC = T // NCORES          # 1024 tokens per core
NT = TPC // 128            # 8 token tiles per core
FCH = F // 128             # 16 F chunks (contraction)
ER = E * R                 # 256
ERCH = ER // 128           # 2 er chunks
WCOLS = ER + E             # 272: [z cols | sim cols]

F16 = mybir.dt.float16
F32 = mybir.dt.float32

_CACHE = {}

LAST_RESULTS = None  # BassKernelResults of the most recent run (for test.py)


def build_nc():
    # Bacc (not raw Bass): its compile passes legalize sync waits to the
    # 1-wait-per-instruction HW limit (move_matmul_waits_to_ldweights +
    # generate_event_semaphores) — raw Bass graphs with recycled PSUM slots
    # fail walrus codegen with "Too many sync wait commands".
    nc = bacc.Bacc(target_bir_lowering=False)

    xT = nc.declare_dram_parameter("xT", [FCH // 4, 128, 4, TPC], F16, isOutput=False)
    W = nc.declare_dram_parameter("W", [128, FCH, WCOLS], F16, isOutput=False)
    Bc = nc.declare_dram_parameter("Bc", [ERCH, 128, O], F16, isOutput=False)
    ident = nc.declare_dram_parameter("ident", [128, 128], F16, isOutput=False)
    out = nc.declare_dram_parameter("out", [TPC, O], F16, isOutput=True)

    XG = 4                     # x chunks per DMA group (8KB runs/partition)
    NG = FCH // XG

    with TileContext(nc) as tc:
        with (
            tc.tile_pool(name="weights", bufs=1) as wpool,
            tc.tile_pool(name="xdata", bufs=1) as xpool,
            tc.tile_pool(name="small", bufs=1) as small,
            tc.tile_pool(name="work", bufs=3) as work,
            tc.tile_pool(name="psum_zs", bufs=2, space="PSUM") as psum_zs,
            tc.tile_pool(name="psum_t", bufs=2, space="PSUM") as psum_t,
            tc.tile_pool(name="psum_d", bufs=4, space="PSUM") as psum_d,
        ):
            # ---- input DMAs: few big ones with >=8KB contiguous runs ----
            w_sb = wpool.tile([128, FCH, WCOLS], F16, name="w_sb")
            nc.sync.dma_start(out=w_sb, in_=W[:, :, :])
            id_sb = wpool.tile([128, 128], F16, name="id_sb")
            nc.sync.dma_start(out=id_sb, in_=ident[:, :])
            bc_sb = wpool.tile([128, ERCH, O], F16, name="bc_sb")
            nc.sync.dma_start(out=bc_sb, in_=Bc[:, :, :].rearrange("c p o -> p c o"))
            x_sb = []
            for g in range(NG):
                xg = xpool.tile([128, XG, TPC], F16, name=f"x_sb{g}", tag=f"x{g}")
                nc.sync.dma_start(out=xg, in_=xT[g])
                x_sb.append(xg)

            # ---- routing stat tiles (written per token tile) ----
            sim_all = small.tile([128, NT, E], F32, name="sim_all")
            m8_all = small.tile([128, NT, 8], F32, name="m8_all")
            m1n_all = small.tile([128, NT], F32, name="m1n_all")
            e_all = small.tile([128, NT, E], F32, name="e_all")
            ge_all = small.tile([128, NT, E], F32, name="ge_all")
            em_all = small.tile([128, NT, E], F32, name="em_all")
            den_all = small.tile([128, NT], F32, name="den_all")
            rcp_all = small.tile([128, NT], F32, name="rcp_all")
            coeff_all = small.tile([128, NT, E], F32, name="coeff_all")

            for i in range(NT):
                # ---- fused z+sim matmul (accumulate over F chunks) ----
                zs = psum_zs.tile([128, WCOLS], F32, name=f"zs{i}", tag="zs")
                for c in range(FCH):
                    nc.tensor.matmul(
                        zs,
                        lhsT=x_sb[c // XG][:, c % XG, ts(i, 128)],
                        rhs=w_sb[:, c, :],
                        start=(c == 0),
                        stop=(c == FCH - 1),
                    )
                # ---- top-4 softmax routing for this tile ----
                sim = sim_all[:, i, :]
                nc.scalar.activation(
                    sim, zs[:, ER:WCOLS], mybir.ActivationFunctionType.Abs
                )
                nc.vector.max(m8_all[:, i, :], sim)
                nc.vector.tensor_scalar_mul(
                    m1n_all[:, i:i + 1], m8_all[:, i, 0:1], -1.0
                )
                nc.scalar.activation(
                    e_all[:, i, :], sim, mybir.ActivationFunctionType.Exp,
                    bias=m1n_all[:, i:i + 1],
                )
                nc.vector.tensor_scalar(
                    ge_all[:, i, :], sim, m8_all[:, i, 3:4], None,
                    op0=mybir.AluOpType.is_ge,
                )
                nc.vector.tensor_tensor(
                    em_all[:, i, :], e_all[:, i, :], ge_all[:, i, :],
                    mybir.AluOpType.mult,
                )
                nc.vector.tensor_reduce(
                    den_all[:, i:i + 1], em_all[:, i, :],
                    axis=mybir.AxisListType.X, op=mybir.AluOpType.add,
                )
                nc.vector.reciprocal(rcp_all[:, i:i + 1], den_all[:, i:i + 1])
                nc.vector.tensor_scalar(
                    coeff_all[:, i, :], em_all[:, i, :], rcp_all[:, i:i + 1],
                    None, op0=mybir.AluOpType.mult,
                )
                # ---- weight z by coeff straight out of PSUM ----
                zw = work.tile([128, E, R], F16, name=f"zw{i}", tag="zw")
                nc.vector.tensor_tensor(
                    zw,
                    zs[:, 0:ER].rearrange("p (e r) -> p e r", r=R),
                    coeff_all[:, i, :, None].to_broadcast([128, E, R]),
                    mybir.AluOpType.mult,
                )
                # ---- transpose zw to put E*R on partitions ----
                zwT = work.tile([128, ERCH, 128], F16, name=f"zwT{i}", tag="zwT")
                for h in range(ERCH):
                    tp = psum_t.tile([128, 128], F16, name=f"tp{i}_{h}", tag="tp")
                    nc.tensor.transpose(
                        tp, zw.rearrange("p e r -> p (e r)")[:, ts(h, 128)], id_sb
                    )
                    nc.scalar.activation(
                        zwT[:, h, :], tp, mybir.ActivationFunctionType.Copy
                    )
                # ---- second matmul against Bcat + store ----
                out_sb = work.tile([128, O], F16, name=f"out_sb{i}", tag="out_sb")
                for q in range(4):
                    dq = psum_d.tile([128, 512], F32, name=f"d{i}_{q}", tag="d")
                    for ch in range(ERCH):
                        nc.tensor.matmul(
                            dq,
                            lhsT=zwT[:, ch, :],
                            rhs=bc_sb[:, ch, ts(q, 512)],
                            start=(ch == 0),
                            stop=(ch == ERCH - 1),
                        )
                    if q % 2 == 0:
                        nc.scalar.activation(
                            out_sb[:, ts(q, 512)], dq,
                            mybir.ActivationFunctionType.Copy,
                        )
                    else:
                        nc.vector.tensor_copy(out=out_sb[:, ts(q, 512)], in_=dq)
                nc.gpsimd.dma_start(out=out[ts(i, 128), :], in_=out_sb)

    nc.finalize()  # runs Bacc.compile(): reg alloc + sync-wait legalization
    return nc


def _host_prep(x, prototypes, A_stack, B_stack, scaling):
    tok = np.ascontiguousarray(x.reshape(T, F))

    Acat = A_stack.reshape(ER, F)
    Wh = np.concatenate([Acat.T, prototypes.T], axis=1)        # (F, 272)
    # W dram layout [128, FCH, WCOLS]: partition-major so the single DMA
    # reads 8.7KB contiguous per partition.
    Wh = np.ascontiguousarray(
        Wh.reshape(FCH, 128, WCOLS).transpose(1, 0, 2)
    ).astype(np.float16)

    Bcat = (B_stack.transpose(0, 2, 1).reshape(ER, O) * float(scaling))
    Bch = Bcat.reshape(ERCH, 128, O).astype(np.float16)

    identh = np.eye(128, dtype=np.float16)

    in_maps = []
    for core in range(NCORES):
        shard = tok[core * TPC:(core + 1) * TPC]               # (TPC, F)
        # xT dram layout [FCH//4, 128, 4, TPC]: groups of 4 F-chunks,
        # partition-major within a group -> 8KB contiguous per partition.
        xTh = (
            shard.T.reshape(FCH // 4, 4, 128, TPC)
            .transpose(0, 2, 1, 3)
            .astype(np.float16)
        )
        in_maps.append({
            "xT": np.ascontiguousarray(xTh),
            "W": Wh,
            "Bc": Bch,
            "ident": identh,
        })
    return in_maps


def _setup_axon_tracing():
    """Make trace=True work in this container: register the NTFF profile
    hook that the image's antenv lacks, and neuter upload_artifacts (no
    artifact store here). Best-effort — failures just disable tracing."""
    import sys
    import types

    import concourse.bass_utils as bu

    bu.upload_artifacts = lambda tmpdir: "local://" + tmpdir
    try:
        from antenv.axon_hooks import get_axon_ntff_profile_hook  # noqa: F401
        return
    except ImportError:
        pass
    import antenv
    from trn_agent_boot.trn_boot import _ntff_profile_via_ctypes

    mod = types.ModuleType("antenv.axon_hooks")
    state = {"hook": _ntff_profile_via_ctypes("/opt/axon/libaxon_pjrt.so")}
    mod.set_axon_ntff_profile_hook = lambda h: state.__setitem__("hook", h)
    mod.get_axon_ntff_profile_hook = lambda: state["hook"]
    antenv.axon_hooks = mod
    sys.modules["antenv.axon_hooks"] = mod


def kernel(x, prototypes, A_stack, B_stack, scaling, top_k):
    global LAST_RESULTS
    assert int(top_k) == TOPK, f"kernel hardcodes top_k={TOPK}, got {top_k}"
    assert x.shape == (B, S, F)

    if "nc" not in _CACHE:
        _CACHE["nc"] = build_nc()
    nc = _CACHE["nc"]

    in_maps = _host_prep(
        np.asarray(x, dtype=np.float32),
        np.asarray(prototypes, dtype=np.float32),
        np.asarray(A_stack, dtype=np.float32),
        np.asarray(B_stack, dtype=np.float32),
        np.asarray(scaling, dtype=np.float32),
    )

    trace = os.environ.get("KERNEL_TRACE", "0") == "1"
    if trace:
        try:
            _setup_axon_tracing()
        except Exception as e:  # tracing is optional; never fail the run
            print(f"tracing setup failed ({e}); running without trace")
            trace = False
    res = run_bass_kernel_spmd(nc, in_maps, core_ids=list(range(NCORES)), trace=trace)
    LAST_RESULTS = res

    outs = [res.results[i]["out"] for i in range(NCORES)]
    full = np.concatenate(outs, axis=0).astype(np.float32)
    return full.reshape(B, S, O)
